# revision 32
# baseline (speedup 1.0000x reference)
"""Binary position embedding kernel for Trainium2, 8-core data-parallel.

out[t, :] = sum_b bit_b(x[t]) * weight[b, :]  ==  bits(x) @ weight

Sharding: x flat [32768] -> 8 shards of 4096 tokens; weight replicated
(host also pre-splits it into bf16 hi/lo rows and replicates into the
2-row-group SBUF layout, a lossless re-encoding of the tiny table).

Per-core plan (4096 tokens -> [4096, 1024] f32 = 16 MiB output; the kernel is
bound by writing that at ~360-400 GB/s, so everything else must hide under
the output-DMA stream and the prologue must be minimal):
  - ONE staged input blob [64, 5124] int16 per core, DMA'd in 2 chunks that
    are hoisted before the kernel's entry barrier so the transfers overlap
    the fixed ~7 us NRT preamble. Blob columns per partition p (r = p % 32):
      [0:1024]    wstack row bitcast: bf16 hi row (r<13) / lo row (13<=r<26)
                  of weight, zeros for gap rows 26..31
      [1024]      bit mask: 1 << (r % 13), 0 for gap rows
      [1028:5124] x broadcast (int16; values < 8192 fit exactly)
  - bits[p, t] = (x[t] & mask[p]) != 0 in bf16 via two DVE ops per chunk
    (bitwise ops can't dtype-cast; the comparison op can).
  - Matmul: per 256-token supertile, 2 row groups run concurrently via
    tile_position=(32g, 0), each contracting K=26 (hi+lo) over its own token
    tile, 2 N-halves of 512. 4 PSUM-bank tags x bufs=2 = 8 banks, so
    consecutive supertiles overlap and the copy/DMA stream never starves.
  - PSUM -> SBUF copies split across DVE and ACT; one 1 MiB HWDGE DMA per
    supertile, alternating between the SP and ACT HWDGE rings.
"""

import numpy as np
import ml_dtypes

import concourse.bass as bass
import concourse.mybir as mybir
from concourse.bass_utils import run_bass_kernel_spmd
from concourse.tile import TileContext
from concourse.vector_clock import ScopedClock


class _LeanTailTileContext(TileContext):
    """Standard tail emits drain -> barrier -> sem clears -> barrier. The
    final barrier only syncs engine-stream ends after the gpsimd-only sem
    clears; dropping it shaves the second EVSEM butterfly off the critical
    path. Re-execution stays safe: clears still run after the full barrier,
    and the next run's entry barrier resynchronizes engines."""

    def _drain_and_barrier(self, tick_clock, wait_clock):
        nc = self.nc
        drain_inst = nc.sync.drain()
        wait_clock.add_sem_waits(
            drain_inst.ins, ScopedClock({None: tick_clock.global_clock})
        )
        nc.all_engine_barrier()
        popped = nc._tile_sem_poison_stack.pop()
        assert popped is self._sem_poison
        nc.clear_and_free_semaphores(list(self.sems.allocated().values()))


N_CORES = 8
B, S, D = 4, 8192, 1024
NB = 13                    # bits per position
GK = 2 * NB                # contraction rows per group (hi+lo)
NG = 4                     # row groups packed into the PE array
TOK = (B * S) // N_CORES   # 4096 tokens per core
TILE = 128
ST = NG * TILE             # 256 tokens per supertile
NST = TOK // ST            # 16 supertiles

# blob layout (int16 columns per partition)
W_COLS = D                 # 1024: wstack row (bf16 bits)
MASK_COL = W_COLS          # 1 column
X_OFF = W_COLS + 4         # x starts here (pad to 4-col alignment)
BLOB_COLS = X_OFF + TOK    # 5124
NPART = 32 * NG            # 64 partitions used
CHUNK0 = 128
CHUNKS = [128, 128, 128, 128, 512, 1024, 2048]

TRACE = False
LAST_RESULTS = None

_wsplit_counter = [0]


def _split_multi_waits(nc):
    """This env's walrus allows only one sync-wait per instruction. Hoist
    extra semaphore waits onto single-wait NoOps inserted just before the
    instruction on the same engine stream (same per-engine program order,
    identical blocking semantics)."""
    import bass_rust

    n_split = 0
    for f in nc.m.functions:
        for bb in f.blocks:
            insts = bb.instructions
            i = 0
            while i < len(insts):
                ins = insts[i]
                si = ins.sync_info
                if si is not None:
                    waits = list(si.on_wait)
                    sem_waits = [w for w in waits if w.sync_type == "semaphore"]
                    other = [w for w in waits if w.sync_type != "semaphore"]
                    keep = 1 if not other else 0
                    if len(waits) > 1 and len(sem_waits) > keep:
                        hoist = sem_waits[: len(sem_waits) - keep]
                        kept = sem_waits[len(sem_waits) - keep:]
                        si.on_wait = other + kept
                        for w in hoist:
                            noop = mybir.InstNoOp(
                                name=f"wsplit-{_wsplit_counter[0]}", ins=[], outs=[]
                            )
                            _wsplit_counter[0] += 1
                            noop.engine = ins.engine
                            noop.sync_info = bass_rust.SyncInfo(
                                on_wait=[w], on_update=[]
                            )
                            insts.insert(i, noop)
                            i += 1
                            n_split += 1
                i += 1
    return n_split


def _drop_entry_barrier(nc):
    """Remove the Tile entry barrier (per-engine Drain + EVSEM butterfly) from
    the preamble block. The preamble's RegisterMoves are same-engine/program-
    order with the body, its memset'd const tiles have no readers, and every
    real cross-engine dependency in the body is semaphore-gated, so the
    barrier only adds latency (~0.2-0.5 us on the critical engine)."""
    main = nc.m.functions[0].blocks[0]
    insts = main.instructions
    i, n = 0, 0
    while i < len(insts):
        ins = insts[i]
        if ins.opcode == "Drain" or ins.name.startswith("barrier_"):
            insts.pop(i)
            n += 1
        else:
            i += 1
    return n


def _hoist_to_preamble(nc, names):
    """Move the named (wait-free) instructions from the body block to the
    preamble block, before the Tile entry barrier, so their DMA transfers
    overlap the fixed kernel-start overhead."""
    main_bb = nc.m.functions[0].blocks[0]
    moved = []
    for f in nc.m.functions:
        for bb in f.blocks:
            if bb is main_bb:
                continue
            insts = bb.instructions
            i = 0
            while i < len(insts):
                if insts[i].name in names:
                    moved.append(insts.pop(i))
                else:
                    i += 1
    pos = 0
    mi = main_bb.instructions
    while pos < len(mi) and mi[pos].opcode in ("Call", "RegisterMove"):
        pos += 1
    for j, ins in enumerate(moved):
        mi.insert(pos + j, ins)
    return len(moved)


def _build():
    f32, bf16 = mybir.dt.float32, mybir.dt.bfloat16
    i16 = mybir.dt.int16
    op = mybir.AluOpType

    nc = bass.Bass()
    blob = nc.declare_dram_parameter("blob", [NPART, BLOB_COLS], i16, isOutput=False)
    out = nc.declare_dram_parameter("out", [TOK, D], f32, isOutput=True)

    hoist_names = []
    with _LeanTailTileContext(nc) as tc:
        with (
            tc.tile_pool(name="const", bufs=1) as cpool,
            tc.tile_pool(name="outp", bufs=8) as opool,
            tc.tile_pool(name="psum", bufs=1, space="PSUM") as ppool,
        ):
            sb = cpool.tile([NPART, BLOB_COLS], i16)
            bits_i = cpool.tile([NPART, TOK], i16)
            bitsT = cpool.tile([NPART, TOK], bf16)

            wstack = sb[:, 0:W_COLS].bitcast(bf16)
            mks = sb[:, MASK_COL : MASK_COL + 1]

            # input DMAs (hoisted to the preamble by name below)
            d0 = nc.scalar.dma_start(
                sb[:, 0 : X_OFF + CHUNK0], blob[:, 0 : X_OFF + CHUNK0]
            )
            d1 = nc.scalar.dma_start(
                sb[:, X_OFF + CHUNK0 :], blob[:, X_OFF + CHUNK0 :]
            )
            hoist_names = [d0.ins.name, d1.ins.name]

            # bits: (x & mask) != 0 -> bf16 (two DVE ops per chunk)
            off = 0
            for cl in CHUNKS:
                xsl = sb[:, X_OFF + off : X_OFF + off + cl]
                nc.vector.tensor_scalar(
                    bits_i[:, off : off + cl], xsl, mks, None, op.bitwise_and
                )
                nc.vector.tensor_scalar(
                    bitsT[:, off : off + cl],
                    bits_i[:, off : off + cl],
                    0,
                    None,
                    op.not_equal,
                )
                off += cl

            # main loop: supertiles of NG*128 tokens
            for s in range(NST):
                ob = opool.tile([TILE, NG * D], f32)
                for g in range(NG):
                    t0 = (s * NG + g) * TILE
                    for h in range(2):
                        pt = ppool.tile([TILE, 512], f32, tag=f"p{g}{h}")
                        nc.tensor.matmul(
                            pt[:],
                            bitsT[32 * g : 32 * g + GK, t0 : t0 + TILE],
                            wstack[32 * g : 32 * g + GK, 512 * h : 512 * (h + 1)],
                            start=True,
                            stop=True,
                            tile_position=(32 * g, 0),
                        )
                        dst = ob[:, g * D + 512 * h : g * D + 512 * (h + 1)]
                        if (g + h) % 2 == 0:
                            nc.vector.tensor_copy(dst, pt[:])
                        else:
                            nc.scalar.copy(dst, pt[:])
                    if s < 2:
                        # start the output stream ASAP: per-tile 512 KiB
                        # DMAs, each issued right after its two copies.
                        # All on SP, which has no other work, so the DVE/ACT
                        # copy streams are never interrupted by issue costs.
                        nc.sync.dma_start(
                            out[t0 : t0 + TILE, :],
                            ob[:, g * D : (g + 1) * D],
                        )
                if s < 2:
                    pass
                else:
                    # one 2 MiB DMA per supertile; alternate HWDGE rings
                    dram_view = out[s * ST : (s + 1) * ST, :].rearrange(
                        "(g p) d -> p g d", p=TILE
                    )
                    eng = nc.sync if s % 2 == 0 else nc.scalar
                    eng.dma_start(
                        dram_view, ob[:].rearrange("p (g d) -> p g d", g=NG)
                    )

    _hoist_to_preamble(nc, set(hoist_names))
    _drop_entry_barrier(nc)
    _split_multi_waits(nc)
    return nc


_nc_cache = None


def _make_blob(xf_shard, weight):
    """Host-staged per-core input blob [NPART, BLOB_COLS] int16."""
    blob = np.zeros((NPART, BLOB_COLS), np.int16)
    w = np.asarray(weight, dtype=np.float32)
    hi = w.astype(ml_dtypes.bfloat16)
    lo = (w - hi.astype(np.float32)).astype(ml_dtypes.bfloat16)
    hi16 = hi.view(np.int16)
    lo16 = lo.view(np.int16)
    for g in range(NG):
        blob[32 * g : 32 * g + NB, 0:W_COLS] = hi16
        blob[32 * g + NB : 32 * g + GK, 0:W_COLS] = lo16
    for p in range(NPART):
        r = p % 32
        if r < GK:
            blob[p, MASK_COL] = 1 << (r % NB)
    blob[:, X_OFF:] = xf_shard[None, :]
    return blob


def kernel(x, weight):
    global _nc_cache, LAST_RESULTS
    if _nc_cache is None:
        _nc_cache = _build()
    nc = _nc_cache

    # x values are < 8192 so they fit int16 exactly
    xf = np.asarray(x, dtype=np.int32).reshape(-1).astype(np.int16)
    in_maps = [
        {"blob": _make_blob(xf[c * TOK : (c + 1) * TOK], weight)}
        for c in range(N_CORES)
    ]
    res = run_bass_kernel_spmd(nc, in_maps, list(range(N_CORES)), trace=TRACE)
    LAST_RESULTS = res
    out = np.concatenate([r["out"] for r in res.results], axis=0)
    return out.reshape(B, S, D)
